# revision 14
# baseline (speedup 1.0000x reference)
"""Causal GQA attention (qk-norm + rope) on 8 TRN2 NeuronCores.

Sharding: tensor-parallel over heads. Core c owns Q heads {2c, 2c+1} and
KV group c//2 (w_qkv column-parallel, w_o row-parallel). Each core
computes a full-shape partial of the output projection; the host sums
the 8 partials (row-parallel w_o => partial sums, no on-device
collective).

Per-core pipeline (all matmuls bf16 on PE, fp32 PSUM accumulate):
  1. qkv = x @ w_qkv_c in natural [s, c] layout (x transposed on PE per
     128-block), then L2 qk-norm (free-dim reduce) + rope, then PE
     transpose of q̂/k̂ into [hd, s] for attention.
  2. Flash-style causal attention per head: S^T[k, q] blocks on PE,
     exp on ACT (scale 1/8 folded in; scores are bounded by +-1/8 after
     qk-norm so no max subtraction), A^T V accumulation on PE with an
     appended ones column producing the softmax denominator for free.
  3. y_partial = out_heads @ w_o_rows, DMA'd out per tile.
"""

import os

import numpy as np
import ml_dtypes

import concourse.bass as bass
import concourse.tile as tile
from concourse import bacc, mybir
from concourse.bass_utils import run_bass_kernel_spmd

F32 = mybir.dt.float32
BF16 = mybir.dt.bfloat16
AF = mybir.ActivationFunctionType
OP = mybir.AluOpType

T = 4096          # sequence length
D = 1024          # d_model
HD = 64           # head dim
NB = T // 128     # 32 seq blocks of 128
NCORES = 8
THETA = 10000.0
NEG = -30000.0    # mask bias; exp(NEG/8) underflows to 0

_built = {}


def _emit(tc, nc, x_d, wqkv_d, wo_d, cs_d, mask_d, id_d, ones_d, y_d):
    with (
        tc.tile_pool(name="pers", bufs=1) as pers,
        tc.tile_pool(name="stage", bufs=2) as stage,
    ):
        # persistent SBUF tensors
        QT0 = pers.tile([64, T], BF16)          # q̂^T head 0
        QT1 = pers.tile([64, T], BF16)          # q̂^T head 1
        KT = pers.tile([64, T], BF16)           # k̂^T
        VT = pers.tile([128, NB, 65], BF16)     # per k-block [V | 1]
        OT = pers.tile([128, T], BF16)          # normalized attn out^T (2 heads)
        wqkv_b = pers.tile([128, 8, 256], BF16)
        wo_b = pers.tile([128, D], BF16)
        cs_sb = pers.tile([128, NB, 64], F32)   # cos|sin per seq block
        mask_sb = pers.tile([128, 128], F32)
        id_sb = pers.tile([128, 128], BF16)
        ones_sb = pers.tile([1, 64], BF16)

        wqkv_f = stage.tile([128, 8, 256], F32, tag="wq_f")
        wo_f = stage.tile([128, D], F32, tag="wo_f")
        nc.sync.dma_start(wqkv_f[:], wqkv_d.rearrange("(j p) c -> p j c", p=128))
        nc.sync.dma_start(wo_f[:], wo_d[:])
        nc.vector.tensor_copy(wqkv_b[:], wqkv_f[:])
        nc.vector.tensor_copy(wo_b[:], wo_f[:])
        nc.sync.dma_start(cs_sb[:], cs_d.rearrange("(n p) c -> p n c", p=128))
        nc.sync.dma_start(mask_sb[:], mask_d[:])
        nc.sync.dma_start(id_sb[:], id_d[:])
        nc.sync.dma_start(ones_sb[:], ones_d[:])
        nc.vector.memset(VT[:, :, 64], 1.0)

        # ---- phase 1: qkv projection + qk-norm + rope, per 128-row block
        with (
            tc.tile_pool(name="p1io", bufs=2) as p1io,
            tc.tile_pool(name="p1w", bufs=2) as p1w,
            tc.tile_pool(name="p1ps", bufs=2, space="PSUM") as p1ps,
            tc.tile_pool(name="p1pt", bufs=2, space="PSUM") as p1pt,
            tc.tile_pool(name="p1pq", bufs=1, space="PSUM") as p1pq,
        ):
            P1 = int(os.environ.get("K_P1", "4"))
            for sb in range(NB):
                x_f = p1io.tile([128, D], F32, tag="x_f")
                nc.sync.dma_start(x_f[:], x_d[sb * 128:(sb + 1) * 128, :])
                x_b = p1io.tile([128, D], BF16, tag="x_b")
                nc.vector.tensor_copy(x_b[:], x_f[:])

                xT = p1w.tile([128, 8, 128], BF16, tag="xT")
                for j in range(8):
                    pt = p1pt.tile([128, 128], BF16, tag="pt")
                    nc.tensor.transpose(pt[:], x_b[:, j * 128:(j + 1) * 128], id_sb[:])
                    nc.scalar.copy(xT[:, j, :], pt[:])

                if P1 < 2:
                    continue
                qkvp = p1ps.tile([128, 256], F32, tag="qkvp")
                for j in range(8):
                    nc.tensor.matmul(qkvp[:], xT[:, j, :], wqkv_b[:, j, :],
                                     start=(j == 0), stop=(j == 7))

                # v slice straight to VT (no norm/rope)
                nc.vector.tensor_copy(VT[:, sb, 0:64], qkvp[:, 192:256])

                qk_s = p1w.tile([128, 192], F32, tag="qk_s")
                nc.vector.tensor_copy(qk_s[:], qkvp[:, 0:192])

                if P1 < 3:
                    continue
                sq = p1w.tile([128, 64], F32, tag="sq")
                sg = p1w.tile([128, 64], F32, tag="sg")
                ss = p1w.tile([128, 4], F32, tag="ss")
                for h3 in range(3):
                    seg = qk_s[:, h3 * 64:(h3 + 1) * 64]
                    nc.vector.tensor_copy(sg[:], seg)
                    nc.vector.tensor_mul(sq[:], sg[:], seg)
                    nc.vector.reduce_sum(ss[:, h3:h3 + 1], sq[:],
                                         axis=mybir.AxisListType.X)
                srt = p1w.tile([128, 4], F32, tag="srt")
                nc.scalar.sqrt(srt[:, 0:3], ss[:, 0:3])
                invn = p1w.tile([128, 4], F32, tag="invn")
                nc.vector.reciprocal(invn[:, 0:3], srt[:, 0:3])

                if P1 < 4:
                    continue
                qhat = p1w.tile([128, 192], BF16, tag="qhat")
                tmp = p1w.tile([128, 64], F32, tag="tmp")
                r1 = p1w.tile([128, 32], F32, tag="r1")
                r2 = p1w.tile([128, 32], F32, tag="r2")
                cosb = cs_sb[:, sb, 0:32]
                sinb = cs_sb[:, sb, 32:64]
                for h3 in range(3):
                    c0 = h3 * 64
                    nc.vector.tensor_scalar_mul(tmp[:], qk_s[:, c0:c0 + 64],
                                                invn[:, h3:h3 + 1])
                    t1, t2 = tmp[:, 0:32], tmp[:, 32:64]
                    nc.vector.tensor_mul(r1[:], t1, cosb)
                    nc.vector.tensor_mul(r2[:], t2, sinb)
                    nc.vector.tensor_sub(qhat[:, c0:c0 + 32], r1[:], r2[:])
                    nc.vector.tensor_mul(r1[:], t2, cosb)
                    nc.vector.tensor_mul(r2[:], t1, sinb)
                    nc.vector.tensor_add(qhat[:, c0 + 32:c0 + 64], r1[:], r2[:])

                pq = p1pq.tile([128, 128], BF16, tag="pq")
                nc.tensor.transpose(pq[:], qhat[:, 0:128], id_sb[:])
                nc.scalar.copy(QT0[:, sb * 128:(sb + 1) * 128], pq[0:64, :])
                nc.scalar.copy(QT1[:, sb * 128:(sb + 1) * 128], pq[64:128, :])
                pk = p1pq.tile([64, 128], BF16, tag="pk")
                nc.tensor.transpose(pk[:], qhat[:, 128:192], id_sb[:])
                nc.scalar.copy(KT[:, sb * 128:(sb + 1) * 128], pk[:])

        # ---- phase 2: causal attention per head + output projection
        with (
            tc.tile_pool(name="p2s", bufs=2, space="PSUM") as p2s,
            tc.tile_pool(name="p2av", bufs=2, space="PSUM") as p2av,
            tc.tile_pool(name="p2bc", bufs=2, space="PSUM") as p2bc,
            tc.tile_pool(name="p2y", bufs=2, space="PSUM") as p2y,
            tc.tile_pool(name="p2sb", bufs=3) as p2sb,
            tc.tile_pool(name="p2n", bufs=2) as p2n,
        ):
            for qc in range(int(os.environ.get("K_QC", "8"))):  # 512-wide q chunks
                q0 = qc * 512
                for h in range(2):
                    qth = QT0 if h == 0 else QT1
                    av = p2av.tile([65, 512], F32, tag="av")
                    for kb in range(4 * qc):       # full blocks below diagonal
                        sp = p2s.tile([128, 512], F32, tag="sp")
                        nc.tensor.matmul(sp[:], KT[:, kb * 128:(kb + 1) * 128],
                                         qth[:, q0:q0 + 512],
                                         start=True, stop=True)
                        ap = p2sb.tile([128, 512], BF16, tag="ap")
                        nc.scalar.activation(ap[:], sp[:], AF.Exp, scale=0.125)
                        nc.tensor.matmul(av[:], VT[:, kb, :], ap[:],
                                         start=(kb == 0), stop=False,
                                         skip_group_check=True)
                    for i in range(4):             # diagonal window blocks
                        kb = 4 * qc + i
                        w = 512 - 128 * i
                        sp = p2s.tile([128, 512], F32, tag="sp")
                        nc.tensor.matmul(sp[:, 0:w], KT[:, kb * 128:(kb + 1) * 128],
                                         qth[:, q0 + 128 * i:q0 + 512],
                                         start=True, stop=True)
                        nc.vector.tensor_add(sp[:, 0:128], sp[:, 0:128], mask_sb[:])
                        ap = p2sb.tile([128, 512], BF16, tag="ap")
                        nc.scalar.activation(ap[:, 0:w], sp[:, 0:w], AF.Exp,
                                             scale=0.125)
                        nc.tensor.matmul(av[:, 128 * i:512], VT[:, kb, :],
                                         ap[:, 0:w],
                                         start=(qc == 0 and i == 0), stop=(i == 3),
                                         skip_group_check=True)
                    # normalize: row 64 of av is the softmax denominator
                    rec = p2n.tile([1, 512], F32, tag="rec")
                    nc.vector.reciprocal(rec[:], av[64:65, :])
                    recb = p2n.tile([1, 512], BF16, tag="recb")
                    nc.vector.tensor_copy(recb[:], rec[:])
                    bc = p2bc.tile([64, 512], F32, tag="bc")
                    nc.tensor.matmul(bc[:], ones_sb[:], recb[:],
                                     start=True, stop=True)
                    bcs = p2n.tile([64, 512], F32, tag="bcs")
                    nc.vector.tensor_copy(bcs[:], bc[:])
                    nc.vector.tensor_mul(OT[64 * h:64 * h + 64, q0:q0 + 512],
                                         av[0:64, :], bcs[:])
                # output projection for this q chunk (both heads ready)
                for qb in range(4):
                    ot_blk = OT[:, q0 + qb * 128:q0 + (qb + 1) * 128]
                    for nh in range(2):
                        yp = p2y.tile([128, 512], F32, tag="yp")
                        nc.tensor.matmul(yp[:], ot_blk, wo_b[:, nh * 512:(nh + 1) * 512],
                                         start=True, stop=True)
                        ys = p2sb.tile([128, 512], F32, tag="ys")
                        nc.vector.tensor_copy(ys[:], yp[:])
                        nc.sync.dma_start(
                            y_d[q0 + qb * 128:q0 + (qb + 1) * 128,
                                nh * 512:(nh + 1) * 512], ys[:])


def _build():
    if "nc" in _built:
        return _built["nc"]
    nc = bacc.Bacc("TRN2", target_bir_lowering=False, debug=False)
    x_d = nc.dram_tensor("x", [T, D], F32, kind="ExternalInput").ap()
    wqkv_d = nc.dram_tensor("wqkv", [D, 256], F32, kind="ExternalInput").ap()
    wo_d = nc.dram_tensor("wo", [128, D], F32, kind="ExternalInput").ap()
    cs_d = nc.dram_tensor("cossin", [T, 64], F32, kind="ExternalInput").ap()
    mask_d = nc.dram_tensor("mask", [128, 128], F32, kind="ExternalInput").ap()
    id_d = nc.dram_tensor("ident", [128, 128], BF16, kind="ExternalInput").ap()
    ones_d = nc.dram_tensor("ones64", [1, 64], BF16, kind="ExternalInput").ap()
    y_d = nc.dram_tensor("y", [T, D], F32, kind="ExternalOutput").ap()
    with tile.TileContext(nc) as tc:
        _emit(tc, nc, x_d, wqkv_d, wo_d, cs_d, mask_d, id_d, ones_d, y_d)
    nc.compile()
    _built["nc"] = nc
    return nc


def host_inputs(x, w_qkv, w_o):
    """Per-core input dicts (shards + constant tables)."""
    x2 = np.ascontiguousarray(np.asarray(x, np.float32).reshape(T, D))
    w_qkv = np.asarray(w_qkv, np.float32)
    w_o = np.asarray(w_o, np.float32)

    half = HD // 2
    inv_freq = 1.0 / (THETA ** (np.arange(half, dtype=np.float32) / half))
    ang = np.arange(T, dtype=np.float32)[:, None] * inv_freq[None, :]
    cossin = np.ascontiguousarray(
        np.concatenate([np.cos(ang), np.sin(ang)], axis=1).astype(np.float32))

    kl = np.arange(128)[:, None]
    ql = np.arange(128)[None, :]
    mask = np.where(ql >= kl, 0.0, NEG).astype(np.float32)
    ident = np.eye(128, dtype=ml_dtypes.bfloat16)
    ones64 = np.ones((1, 64), dtype=ml_dtypes.bfloat16)

    maps = []
    for c in range(NCORES):
        g = c // 2
        wq = np.ascontiguousarray(np.concatenate([
            w_qkv[:, 128 * c:128 * c + 128],          # 2 q heads
            w_qkv[:, 1024 + 64 * g:1024 + 64 * g + 64],   # k group
            w_qkv[:, 1280 + 64 * g:1280 + 64 * g + 64],   # v group
        ], axis=1))
        wo_c = np.ascontiguousarray(w_o[128 * c:128 * c + 128, :])
        maps.append(dict(x=x2, wqkv=wq, wo=wo_c, cossin=cossin, mask=mask,
                         ident=ident, ones64=ones64))
    return maps


def kernel(x, w_qkv, w_o):
    nc = _build()
    maps = host_inputs(x, w_qkv, w_o)
    res = run_bass_kernel_spmd(nc, maps, list(range(NCORES))).results
    y = np.zeros((T, D), np.float64)
    for c in range(NCORES):
        y += np.asarray(res[c]["y"], np.float64)
    return y.astype(np.float32).reshape(1, T, D)
